# revision 4
# baseline (speedup 1.0000x reference)
"""Complex-valued fully-connected layer on 8 TRN2 NeuronCores.

Math (per reference):
    out_re = x_re @ w_re^T - x_im @ w_im^T
    out_im = x_re @ w_im^T + x_im @ w_re^T        -> stack([out_re, out_im])
with x_*: [8192, 2048] f32, w_*: [2048, 2048] f32.

Strategy:
  - Shard 8 cores = 2 batch-halves (4096 rows) x 4 out-feature quarters (512).
    Outputs are disjoint -> no collectives.
  - Karatsuba (3 real GEMMs instead of 4):
        t_rr = x_re @ w_re^T ; t_ii = x_im @ w_im^T ; t_ss = (x_re+x_im)@(w_re+w_im)^T
        out_re = t_rr - t_ii ; out_im = t_ss - t_rr - t_ii
    x_s/w_s are formed on-device by DVE (cheap adds, off the critical path).
  - bf16 inputs (fp32 PSUM accumulate): same PE rate as float32r (1 row/cycle
    at free>=256) but half the DMA bytes. x is DMA'd in 256-batch-column
    chunks so every descriptor moves >=512B contiguous runs (the DMA engines
    pay 2x below 512B).
  - Raw bass, explicit semaphores, two HWDGE rings (SP: x chunks, ACT:
    weights + output stores). 4-chunk (8-tile) x buffering keeps DMA
    chunks ahead of PE; PE waits are minimal and never gate on a
    DVE-produced semaphore where a DMA one suffices (that coupling cost
    26us/iter on HW), so the PE p-state ramp never resets mid-run (a
    stall means 2x slower matmuls for the next 3us). DVE emits the
    PE-gating combines before any DMA-waiting op.
  - Prologue: tile order rr0,ii0,rr1,ii1,ss0,ss1 with k-half split w/x
    loads and k-half w_s adds hides the weight-load latency behind the
    first matmul groups.
"""

import numpy as np
import ml_dtypes

import concourse.bass as bass
from concourse import mybir
from concourse.bass_utils import run_bass_kernel_spmd

BATCH, IN_F, OUT_F = 8192, 2048, 2048
N_CORES = 8
B_SHARDS, O_SHARDS = 2, 4
B_SH = BATCH // B_SHARDS          # 4096 batch rows per core
O_SH = OUT_F // O_SHARDS          # 512 out features per core
KT = IN_F // 128                  # 16 contraction tiles
BT = B_SH // 128                  # 32 batch tiles per core
NCH = BT // 2                     # 16 x-chunks (256 batch cols each)
XBUF = 4                          # x chunk buffers in SBUF

F32 = mybir.dt.float32
BF16 = mybir.dt.bfloat16
BF16_NP = ml_dtypes.bfloat16


def build_nc(
    repeat: int = 1,
    prologue_split: bool = True,
    tail_split: bool = True,
    decouple_sx: bool = True,
    cmb_first: bool = True,
    deep_psum: bool = False,
    xbuf: int = XBUF,
) -> bass.Bass:
    """repeat > 1 re-runs the whole pipeline on the same data (timing only)."""
    XBUF = xbuf  # shadow module default; all body refs use this
    nc = bass.Bass("TRN2", target_bir_lowering=False, debug=False)

    xt_re = nc.dram_tensor("xt_re", [IN_F, B_SH], BF16, kind="ExternalInput")
    xt_im = nc.dram_tensor("xt_im", [IN_F, B_SH], BF16, kind="ExternalInput")
    wt_re = nc.dram_tensor("wt_re", [IN_F, O_SH], BF16, kind="ExternalInput")
    wt_im = nc.dram_tensor("wt_im", [IN_F, O_SH], BF16, kind="ExternalInput")
    out_d = nc.dram_tensor("out", [2, B_SH, O_SH], F32, kind="ExternalOutput")

    # SBUF: weights resident [p, k, o]; x chunks [p, buf, k, 256] (one chunk
    # = two 128-row PE tiles, 512B contiguous runs); output staging holds a
    # tile-pair [p, pairbuf, c, subtile, o]; r_sb stages t_rr (PSUM->SBUF).
    w_re_sb = nc.alloc_sbuf_tensor("w_re_sb", [128, KT, O_SH], BF16)
    w_im_sb = nc.alloc_sbuf_tensor("w_im_sb", [128, KT, O_SH], BF16)
    w_s_sb = nc.alloc_sbuf_tensor("w_s_sb", [128, KT, O_SH], BF16)
    x_re_sb = nc.alloc_sbuf_tensor("x_re_sb", [128, XBUF, KT, 256], BF16)
    x_im_sb = nc.alloc_sbuf_tensor("x_im_sb", [128, XBUF, KT, 256], BF16)
    x_s_sb = nc.alloc_sbuf_tensor("x_s_sb", [128, XBUF, KT, 256], BF16)
    o_sb = nc.alloc_sbuf_tensor("o_sb", [128, 2, 2, 2, O_SH], F32)
    r_sb = nc.alloc_sbuf_tensor("r_sb", [128, 2, O_SH], F32)

    # deep_psum: rr/ii 3-deep + ss 2-deep = all 8 banks; tile-start reuse
    # wait relaxes to cmb_done >= T-2 with the hard >= T-1 wait deferred to
    # the ss group (cushion against DVE-combine jitter).
    RD = 3 if deep_psum else 2
    p_rr = [nc.alloc_psum_tensor(f"p_rr{b}", [128, O_SH], F32) for b in range(RD)]
    p_ii = [nc.alloc_psum_tensor(f"p_ii{b}", [128, O_SH], F32) for b in range(RD)]
    p_ss = [nc.alloc_psum_tensor(f"p_ss{b}", [128, O_SH], F32) for b in range(2)]

    # DRAM views: contraction dim folded to [partition, ktile, cols]; output
    # folded so one store covers a 256-row tile-pair per component.
    xt_re_r = xt_re.ap().rearrange("(k p) b -> p k b", p=128)
    xt_im_r = xt_im.ap().rearrange("(k p) b -> p k b", p=128)
    wt_re_r = wt_re.ap().rearrange("(k p) o -> p k o", p=128)
    wt_im_r = wt_im.ap().rearrange("(k p) o -> p k o", p=128)
    out_r = out_d.ap().rearrange("c (j s p) o -> p c j s o", p=128, s=2)
    out_t = out_d.ap().rearrange("c (t p) o -> p c t o", p=128)

    TT = BT * repeat              # total tiles over all repeats
    TCH = NCH * repeat            # total chunks

    # mm_done increments per matmul group. The prologue interleaves tiles
    # 0/1 as rr0,ii0,rr1,ii1,ss0,ss1, so cmb thresholds for T=0/1 differ
    # from the steady-state 3T+1/3T+2/3T+3 pattern.
    CMB_W = {0: (1, 2, 5), 1: (3, 4, 6)}

    def cmb_waits(T):
        return CMB_W.get(T, (3 * T + 1, 3 * T + 2, 3 * T + 3))

    with (
        nc.Block() as block,
        nc.semaphore("dma_x") as dma_x,      # SP ring: x chunk loads (32/chunk)
        nc.semaphore("dma_w") as dma_w,      # ACT ring: 2 weight loads
        nc.semaphore("sw_done") as sw_done,  # DVE: w_s ready
        nc.semaphore("sx_done") as sx_done,  # DVE: x_s ready (1/chunk)
        nc.semaphore("mm_done") as mm_done,  # PE: 1 inc per matmul group
        nc.semaphore("cmb_done") as cmb_done,  # DVE: 1 inc per tile combined
        nc.semaphore("dma_out") as dma_out,  # ACT ring: out stores (16 each)
    ):

        @block.sync
        def _(sp):
            KH = KT // 2
            for G in range(TCH):
                c = G % NCH
                if G >= XBUF:
                    # buffer of chunk G-XBUF free once both its tiles'
                    # groups all ran (count = 6 per chunk)
                    sp.wait_ge(mm_done, 6 * (G - XBUF + 1))
                if G == 0 and prologue_split:
                    # prologue: k-halves so PE's first groups can start as
                    # soon as the first half-slices land. HWDGE incs must be
                    # multiples of 16, so chunk 0 contributes 64 (not 32) to
                    # dma_x — later thresholds carry a +32 offset.
                    for xsb, xr in ((x_re_sb, xt_re_r), (x_im_sb, xt_im_r)):
                        for h in range(2):
                            sp.dma_start(
                                out=xsb.ap()[:, 0, h * KH:(h + 1) * KH, :],
                                in_=xr[:, h * KH:(h + 1) * KH, 0:256],
                            ).then_inc(dma_x, 16)
                    continue
                if G == 0:
                    sp.dma_start(
                        out=x_re_sb.ap()[:, 0, :, :], in_=xt_re_r[:, :, 0:256]
                    ).then_inc(dma_x, 32)
                    sp.dma_start(
                        out=x_im_sb.ap()[:, 0, :, :], in_=xt_im_r[:, :, 0:256]
                    ).then_inc(dma_x, 32)
                    continue
                sp.dma_start(
                    out=x_re_sb.ap()[:, G % XBUF, :, :],
                    in_=xt_re_r[:, :, c * 256:(c + 1) * 256],
                ).then_inc(dma_x, 16)
                sp.dma_start(
                    out=x_im_sb.ap()[:, G % XBUF, :, :],
                    in_=xt_im_r[:, :, c * 256:(c + 1) * 256],
                ).then_inc(dma_x, 16)

        @block.tensor
        def _(pe):
            def mm_group(T, xs, ws, ps, half_waits=None):
                G, b = T // 2, T % 2
                for k in range(KT):
                    if half_waits is not None and k % (KT // 2) == 0:
                        xw, ww = half_waits[k // (KT // 2)]
                        pe.wait_ge(dma_x, xw)
                        pe.wait_ge(dma_w, ww)
                    mm = pe.matmul(
                        out=ps.ap(),
                        lhsT=xs.ap()[:, G % XBUF, k, b * 128:(b + 1) * 128],
                        rhs=ws.ap()[:, k, :],
                        start=(k == 0),
                        stop=(k == KT - 1),
                    )
                mm.then_inc(mm_done, 1)

            # prologue: cover the w_im load + w_s/x_s DVE adds with the
            # rr/ii groups of tiles 0 and 1 (distinct PSUM banks throughout);
            # k-half waits chase the split first loads as they land
            if prologue_split:
                mm_group(0, x_re_sb, w_re_sb, p_rr[0],  # rr0
                         half_waits=((16, 16), (32, 32)))
                mm_group(0, x_im_sb, w_im_sb, p_ii[0],  # ii0
                         half_waits=((48, 48), (64, 64)))
            else:
                pe.wait_ge(dma_x, 32)
                pe.wait_ge(dma_w, 32)
                mm_group(0, x_re_sb, w_re_sb, p_rr[0])  # rr0
                pe.wait_ge(dma_x, 64)
                pe.wait_ge(dma_w, 64)
                mm_group(0, x_im_sb, w_im_sb, p_ii[0])  # ii0
            mm_group(1, x_re_sb, w_re_sb, p_rr[1])      # rr1
            mm_group(1, x_im_sb, w_im_sb, p_ii[1])      # ii1
            pe.wait_ge(sw_done, 1)
            pe.wait_ge(sx_done, 1)
            mm_group(0, x_s_sb, w_s_sb, p_ss[0])        # ss0
            mm_group(1, x_s_sb, w_s_sb, p_ss[1])        # ss1

            for T in range(2, TT):
                G, b = T // 2, T % 2
                if b == 0:
                    if decouple_sx:
                        # rr/ii need only the chunk DMA; gating them on
                        # sx_done would couple tile starts to DVE latency
                        pe.wait_ge(dma_x, 32 * (G + 1) + 32)
                    else:
                        pe.wait_ge(sx_done, G + 1)
                if deep_psum:
                    if T >= 3:
                        pe.wait_ge(cmb_done, T - 2)
                else:
                    pe.wait_ge(cmb_done, T - 1)
                mm_group(T, x_re_sb, w_re_sb, p_rr[T % RD])
                mm_group(T, x_im_sb, w_im_sb, p_ii[T % RD])
                if decouple_sx and b == 0:
                    pe.wait_ge(sx_done, G + 1)
                if deep_psum:
                    pe.wait_ge(cmb_done, T - 1)
                mm_group(T, x_s_sb, w_s_sb, p_ss[b])

        @block.vector
        def _(dve):
            def sx(G):
                # +32 offset: chunk 0's four split loads inc 16 each
                dve.wait_ge(dma_x, 32 * (G + 1) + 32)
                dve.tensor_add(
                    x_s_sb.ap()[:, G % XBUF, :, :],
                    x_re_sb.ap()[:, G % XBUF, :, :],
                    x_im_sb.ap()[:, G % XBUF, :, :],
                ).then_inc(sx_done, 1)

            def cmb(T):
                b = T % 2
                pb = (T // 2) % 2
                w1, w2, w3 = cmb_waits(T)
                if T >= 4:
                    # pair staging reuse: outputs of pair (T//2 - 2) flushed
                    dve.wait_ge(dma_out, 32 * (T // 2 - 1))
                dve.wait_ge(mm_done, w1)
                dve.tensor_copy(r_sb.ap()[:, b, :], p_rr[T % RD].ap())
                dve.wait_ge(mm_done, w2)
                dve.tensor_sub(
                    o_sb.ap()[:, pb, 0, b, :],
                    r_sb.ap()[:, b, :],
                    p_ii[T % RD].ap(),
                )
                dve.wait_ge(mm_done, w3)
                dve.tensor_sub(
                    o_sb.ap()[:, pb, 1, b, :], p_ss[b].ap(), r_sb.ap()[:, b, :]
                )
                dve.tensor_sub(
                    o_sb.ap()[:, pb, 1, b, :],
                    o_sb.ap()[:, pb, 1, b, :],
                    p_ii[T % RD].ap(),
                ).then_inc(cmb_done, 1)

            sx(0)
            dve.wait_ge(dma_w, 64)
            dve.tensor_add(w_s_sb.ap(), w_re_sb.ap(), w_im_sb.ap()).then_inc(
                sw_done, 1
            )
            for G in range(1, TCH):
                # combines first: they gate PE's PSUM reuse, and must never
                # queue behind an sx that is waiting on a DMA
                if cmb_first:
                    cmb(2 * (G - 1))
                    cmb(2 * (G - 1) + 1)
                    sx(G)
                else:
                    sx(G)
                    cmb(2 * (G - 1))
                    cmb(2 * (G - 1) + 1)
            cmb(2 * TCH - 2)
            cmb(2 * TCH - 1)

        @block.scalar
        def _(act):
            KH = KT // 2
            for wsb, wr in ((w_re_sb, wt_re_r), (w_im_sb, wt_im_r)):
                for h in range(2):
                    act.dma_start(
                        out=wsb.ap()[:, h * KH:(h + 1) * KH, :],
                        in_=wr[:, h * KH:(h + 1) * KH, :],
                    ).then_inc(dma_w, 16)
            for J in range(TT // 2):
                if J == TT // 2 - 1 and tail_split:
                    # tail: per-tile stores so tile 2J's flush overlaps the
                    # last tile's matmuls/combine
                    for s in range(2):
                        act.wait_ge(cmb_done, 2 * J + 1 + s)
                        for c in range(2):
                            act.dma_start(
                                out=out_t[:, c, (2 * J + s) % BT, :],
                                in_=o_sb.ap()[:, J % 2, c, s, :],
                            ).then_inc(dma_out, 16)
                    continue
                act.wait_ge(cmb_done, 2 * J + 2)
                for c in range(2):
                    act.dma_start(
                        out=out_r[:, c, J % NCH, :, :],
                        in_=o_sb.ap()[:, J % 2, c, :, :],
                    ).then_inc(dma_out, 16)

    return nc


_NC = None
_RUN_KWARGS: dict = {}   # test.py sets {"trace": True} for profiling runs
LAST_RES = None          # last BassKernelResults


def _get_nc() -> bass.Bass:
    global _NC
    if _NC is None:
        _NC = build_nc()
    return _NC


def prepare_in_maps(x_re, x_im, w_re, w_im):
    x_re = np.asarray(x_re, dtype=np.float32)
    x_im = np.asarray(x_im, dtype=np.float32)
    w_re = np.asarray(w_re, dtype=np.float32)
    w_im = np.asarray(w_im, dtype=np.float32)

    def xh(x):  # batch-half h, transposed to [in, b], bf16
        return [
            np.ascontiguousarray(x[h * B_SH:(h + 1) * B_SH, :].T).astype(BF16_NP)
            for h in range(B_SHARDS)
        ]

    def wq(w):  # out-feature quarter q, transposed to [in, o], bf16
        wt = w.T
        return [
            np.ascontiguousarray(wt[:, q * O_SH:(q + 1) * O_SH]).astype(BF16_NP)
            for q in range(O_SHARDS)
        ]

    xt_re_h, xt_im_h = xh(x_re), xh(x_im)
    wt_re_q, wt_im_q = wq(w_re), wq(w_im)

    in_maps = []
    for c in range(N_CORES):
        bs, os_ = c // O_SHARDS, c % O_SHARDS
        in_maps.append(
            {
                "xt_re": xt_re_h[bs],
                "xt_im": xt_im_h[bs],
                "wt_re": wt_re_q[os_],
                "wt_im": wt_im_q[os_],
            }
        )
    return in_maps


def kernel(x_re, x_im, w_re, w_im):
    in_maps = prepare_in_maps(x_re, x_im, w_re, w_im)
    nc = _get_nc()
    res = run_bass_kernel_spmd(
        nc, in_maps, core_ids=list(range(N_CORES)), **_RUN_KWARGS
    )
    global LAST_RES
    LAST_RES = res

    out = np.empty((2, BATCH, OUT_F), dtype=np.float32)
    for c in range(N_CORES):
        bs, os_ = c // O_SHARDS, c % O_SHARDS
        out[:, bs * B_SH:(bs + 1) * B_SH, os_ * O_SH:(os_ + 1) * O_SH] = (
            res.results[c]["out"]
        )
    return out

